# revision 7
# baseline (speedup 1.0000x reference)
"""KroneckerLSTM trn2 kernel.

Computes, for 8 gate-klins (L @ t @ R + b, t in {x,h}):
    i = sigmoid(klin_ii(x) + klin_hi(h)); f = sigmoid(...); g = tanh(...); o = sigmoid(...)
    c_new = f*c + i*g ; h_new = o*tanh(c_new)
Returns (h_new, c_new), each [1024,1024] f32.

Sharding: output rows split across 8 cores (128 rows each) -> zero collectives.
Per core, for each gate g:  B_g[rows,:] = (L_g[rows,:] @ t) @ R_g
  mm1 computes A^T directly (lhsT = t tiles (natural), rhs = host-pretransposed
  L^T column-slices, stacked 4 gates wide so N=512), so mm1's PSUM output is the
  lhsT for mm2 (rhs = R_g in natural layout).  The x-klin and h-klin of each
  gate pair accumulate into the same PSUM bank; bias is added in-place in PSUM.
"""

import sys

import numpy as np

if "/opt/trn_rl_repo" not in sys.path:
    sys.path.insert(0, "/opt/trn_rl_repo")

N = 1024
M = 1024
P = 128
NC = 8
KT = N // P  # 8 k-tiles of 128
# gate pairs in order i, f, g, o: (x-gate, h-gate, activation)
PAIRS = [("ii", "hi", "Sigmoid"), ("if", "hf", "Sigmoid"),
         ("ig", "hg", "Tanh"), ("io", "ho", "Sigmoid")]

_cache = {}
MM_DTYPE = "fp32r"  # "fp32r" (fast, ~1.5e-4 rel) or "fp32" (~3e-7 rel)


def _build_program():
    import concourse.bass as bass
    import concourse.mybir as mybir
    import concourse.tile as tile
    from concourse import bacc
    from concourse.bass import ts

    FP = mybir.dt.float32
    FIN = mybir.dt.float32r if MM_DTYPE == "fp32r" else FP
    AF = mybir.ActivationFunctionType

    nc = bacc.Bacc("TRN2", target_bir_lowering=False, debug=False,
                   enable_asserts=False, num_devices=NC)

    x_d = nc.dram_tensor("x", [N, M], FIN, kind="ExternalInput").ap()
    h_d = nc.dram_tensor("h", [N, M], FIN, kind="ExternalInput").ap()
    ltx_d = nc.dram_tensor("ltx", [N, 4 * P], FIN, kind="ExternalInput").ap()
    lth_d = nc.dram_tensor("lth", [N, 4 * P], FIN, kind="ExternalInput").ap()
    rx_d = [nc.dram_tensor(f"rx{p}", [M, M], FIN, kind="ExternalInput").ap()
            for p in range(4)]
    rh_d = [nc.dram_tensor(f"rh{p}", [M, M], FIN, kind="ExternalInput").ap()
            for p in range(4)]
    bs_d = nc.dram_tensor("bsum", [4 * P, M], FP, kind="ExternalInput").ap()
    c_d = nc.dram_tensor("cprev", [P, M], FP, kind="ExternalInput").ap()
    hn_d = nc.dram_tensor("h_new", [P, M], FP, kind="ExternalOutput").ap()
    cn_d = nc.dram_tensor("c_new", [P, M], FP, kind="ExternalOutput").ap()

    with tile.TileContext(nc) as tc:
        from contextlib import ExitStack
        with ExitStack() as ctx:
            tp = ctx.enter_context(tc.tile_pool(name="tp", bufs=6))
            ltp = ctx.enter_context(tc.tile_pool(name="ltp", bufs=2))
            atp = ctx.enter_context(tc.tile_pool(name="at", bufs=1))
            rp = ctx.enter_context(tc.tile_pool(name="rstream", bufs=16))
            psp = ctx.enter_context(tc.tile_pool(name="ps", bufs=8, space="PSUM"))
            bsp = ctx.enter_context(tc.tile_pool(name="bsp", bufs=2))
            gp = ctx.enter_context(tc.tile_pool(name="gates", bufs=1))
            ew = ctx.enter_context(tc.tile_pool(name="ew", bufs=1))
            wp = ctx.enter_context(tc.tile_pool(name="warm", bufs=1))

            BF = mybir.dt.bfloat16
            # PE warm-up burst: regular-mode bf16 MMs covering the DMA prologue
            # so HAM is at K=8/8 when the first real (fp32r) matmul issues.
            wa = wp.tile([P, P], BF, tag="wa")
            wb = wp.tile([P, 512], BF, tag="wb")
            nc.vector.memset(wa[:], 0.0)
            nc.vector.memset(wb[:], 0.0)
            wps = psp.tile([P, 512], FP, tag="bank", name="warm_ps")
            for w in range(36):
                nc.tensor.matmul(wps[:], wa[:], wb[:], start=True, stop=True,
                                 skip_group_check=True)

            # c load early (used at the very end)
            cs = ew.tile([P, M], FP, tag="cs")
            nc.sync.dma_start(cs[:], c_d[:])

            # mm1: at_s[j][mloc, 4*128] = sum_k t[k, j*128+mloc] * LT[k, col]
            # kc-outer over j-waves of 4 PSUM banks; t streamed in column
            # halves (wave w uses t[:, 512w:512w+512]); LT resident per stack.
            ats = {"x": [], "h": []}
            for s, t_dram, lt_dram in (("x", x_d, ltx_d), ("h", h_d, lth_d)):
                lts = []
                for kc in range(KT):
                    lt = ltp.tile([P, 4 * P], FIN, tag=f"lt{kc}")
                    nc.sync.dma_start(lt[:], lt_dram[ts(kc, P), :])
                    lts.append(lt)
                for wave in range(2):
                    pts = [psp.tile([P, 4 * P], FP, tag="bank",
                                    name=f"pt_{s}_{wave}_{jj}") for jj in range(4)]
                    for kc in range(KT):
                        th = tp.tile([P, 512], FIN, tag="t")
                        nc.sync.dma_start(th[:], t_dram[ts(kc, P), ts(wave, 512)])
                        for jj in range(4):
                            nc.tensor.matmul(pts[jj][:], th[:, ts(jj, P)], lts[kc][:],
                                             start=(kc == 0), stop=(kc == KT - 1))
                    for jj in range(4):
                        at = atp.tile([P, 4 * P], FIN, tag=f"at{s}{wave * 4 + jj}")
                        nc.vector.tensor_copy(at[:], pts[jj][:])
                        ats[s].append(at)

            # mm2 per gate pair (+ bias + activation):
            # pre[n', m''] = sum_j at[s][j][:, p].T @ R_s[j, :]  over s in {x,h}
            gates = []

            def pair_mm(p, actname):
                pt0 = psp.tile([P, 512], FP, tag="bank", name=f"p{p}b0")
                pt1 = psp.tile([P, 512], FP, tag="bank", name=f"p{p}b1")
                for s, rd in (("x", rx_d[p]), ("h", rh_d[p])):
                    for j in range(KT):
                        rt = rp.tile([P, M], FIN, tag="r")
                        nc.sync.dma_start(rt[:], rd[ts(j, P), :])
                        first = (s == "x") and (j == 0)
                        last = (s == "h") and (j == KT - 1)
                        lhsT = ats[s][j][:, ts(p, P)]
                        nc.tensor.matmul(pt0[:], lhsT, rt[:, 0:512],
                                         start=first, stop=last)
                        nc.tensor.matmul(pt1[:], lhsT, rt[:, 512:1024],
                                         start=first, stop=last)
                bt = bsp.tile([P, M], FP, tag="bs")
                nc.sync.dma_start(bt[:], bs_d[ts(p, P), :])
                nc.vector.tensor_add(pt0[:], pt0[:], bt[:, 0:512])
                nc.vector.tensor_add(pt1[:], pt1[:], bt[:, 512:1024])
                gt = gp.tile([P, M], FP, tag=f"g{p}")
                af = getattr(AF, actname)
                nc.scalar.activation(gt[:, 0:512], pt0[:], af)
                nc.scalar.activation(gt[:, 512:1024], pt1[:], af)
                gates.append(gt)

            for p in range(3):  # i, f, g
                pair_mm(p, PAIRS[p][2])
            gi, gf, gg = gates

            # c_new chain overlaps the o-gate matmuls
            fc = ew.tile([P, M], FP, tag="fc")
            ig = ew.tile([P, M], FP, tag="ig")
            cn = ew.tile([P, M], FP, tag="cn")
            tch = ew.tile([P, M], FP, tag="tch")
            for hf in range(2):
                sl = ts(hf, 512)
                nc.vector.tensor_mul(fc[:, sl], gf[:, sl], cs[:, sl])
                nc.vector.tensor_mul(ig[:, sl], gi[:, sl], gg[:, sl])
                nc.vector.tensor_add(cn[:, sl], fc[:, sl], ig[:, sl])
                nc.sync.dma_start(cn_d[:, sl], cn[:, sl])
                nc.scalar.activation(tch[:, sl], cn[:, sl], AF.Tanh)

            pair_mm(3, PAIRS[3][2])  # o
            go = gates[3]
            hn = ew.tile([P, M], FP, tag="hn")
            for hf in range(2):
                sl = ts(hf, 512)
                nc.vector.tensor_mul(hn[:, sl], go[:, sl], tch[:, sl])
                nc.sync.dma_start(hn_d[:, sl], hn[:, sl])

    nc.compile()
    return nc


def _get_program():
    if "nc" not in _cache:
        _cache["nc"] = _build_program()
    return _cache["nc"]


def _prep_in_maps(inputs):
    f32 = lambda a: np.ascontiguousarray(np.asarray(a, dtype=np.float32))
    x = f32(inputs["x"]); h = f32(inputs["h"]); c = f32(inputs["c"])
    LTx = [f32(np.asarray(inputs[f"L_{xg}"]).T) for xg, _, _ in PAIRS]
    LTh = [f32(np.asarray(inputs[f"L_{hg}"]).T) for _, hg, _ in PAIRS]
    Rx = [f32(inputs[f"R_{xg}"]) for xg, _, _ in PAIRS]
    Rh = [f32(inputs[f"R_{hg}"]) for _, hg, _ in PAIRS]
    bsum = [f32(np.asarray(inputs[f"b_{xg}"]) + np.asarray(inputs[f"b_{hg}"]))
            for xg, hg, _ in PAIRS]

    in_maps = []
    for k in range(NC):
        sl = slice(P * k, P * (k + 1))
        im = {
            "x": x, "h": h,
            "ltx": np.ascontiguousarray(np.concatenate([lt[:, sl] for lt in LTx], axis=1)),
            "lth": np.ascontiguousarray(np.concatenate([lt[:, sl] for lt in LTh], axis=1)),
            "bsum": np.ascontiguousarray(np.concatenate([b[sl] for b in bsum], axis=0)),
            "cprev": np.ascontiguousarray(c[sl]),
        }
        for p in range(4):
            im[f"rx{p}"] = Rx[p]
            im[f"rh{p}"] = Rh[p]
        in_maps.append(im)
    return in_maps


def kernel(**inputs):
    from concourse.bass_utils import run_bass_kernel_spmd

    nc = _get_program()
    in_maps = _prep_in_maps(inputs)
    res = run_bass_kernel_spmd(nc, in_maps, core_ids=list(range(NC)))
    h_new = np.concatenate([res.results[k]["h_new"] for k in range(NC)], axis=0)
    c_new = np.concatenate([res.results[k]["c_new"] for k in range(NC)], axis=0)
    return (h_new, c_new)
